# revision 14
# baseline (speedup 1.0000x reference)
"""CenterLoss kernel for Trainium2 (Bass/Tile), 8-core data-parallel.

loss = mean_b( clip(||x_b - centers[labels_b]||^2, 1e-12, 1e12) )

Sharding: batch (2048) split across 8 cores (256 rows each); centers
replicated (each core only *reads* its 256 gathered rows via indirect
DMA, so the 51MB table costs ~nothing in kernel time). Each core emits
a [1,2] partial-sum pair already scaled by 1/B; the host adds the 16
partials (the cross-core all-reduce) and returns the scalar.

Layout: the host marshals each core's 256 batch rows into a single
[128, 2*128] partition-major tile (row b = n*128+p lands at partition p,
columns n*128..n*128+127) and labels into [128, 2] int32. The kernel
then needs exactly one idx DMA, one x DMA, one indirect gather (256
descriptors), a DVE sub/mul/reduce/clip chain, a PE matmul against a
1/B-scaled ones vector for the partition reduction, and one 8-byte
store. The critical path is dominated by fixed DMA latencies
(SEQ issue + DGE + completion-semaphore propagation), so everything is
merged to pay each fixed cost once.
"""

import numpy as np

import concourse.bacc as bacc
import concourse.bass as bass
import concourse.tile as tile
from concourse import mybir
from concourse.bass_utils import run_bass_kernel_spmd

B, C, D = 2048, 100000, 128
N_CORES = 8
BP = B // N_CORES  # 256 rows per core
P = 128  # SBUF partitions
NT = BP // P  # row-groups per partition (2)
CLAMP_MIN, CLAMP_MAX = 1e-12, 1e12

_NC = None


def _build_nc():
    nc = bacc.Bacc()
    x = nc.declare_dram_parameter("x", [P, NT * D], mybir.dt.float32, isOutput=False)
    idx = nc.declare_dram_parameter("idx", [P, NT], mybir.dt.int32, isOutput=False)
    centers = nc.declare_dram_parameter(
        "centers", [C, D], mybir.dt.float32, isOutput=False
    )
    out = nc.declare_dram_parameter("out", [1, NT], mybir.dt.float32, isOutput=True)

    with tile.TileContext(nc) as tc:
        with (
            tc.tile_pool(name="work", bufs=1) as work,
            tc.tile_pool(name="psum", bufs=1, space="PSUM") as psum,
        ):
            ones = work.tile([P, 1], mybir.dt.float32)
            nc.vector.memset(ones[:], 1.0 / B)

            it = work.tile([P, NT], mybir.dt.int32)
            nc.sync.dma_start(out=it[:], in_=idx[:, :])
            xt = work.tile([P, NT * D], mybir.dt.float32)
            nc.sync.dma_start(out=xt[:], in_=x[:, :])
            # The HW SWDGE offset table holds one offset per partition, so a
            # gather moves at most 128 rows -> one indirect DMA per column
            # block. Splitting also pipelines: block 0's DVE chain overlaps
            # block 1's descriptor generation + transfer.
            ct = work.tile([P, NT * D], mybir.dt.float32)
            diff = work.tile([P, NT * D], mybir.dt.float32)
            sq = work.tile([P, NT * D], mybir.dt.float32)
            dist = work.tile([P, NT], mybir.dt.float32)
            for t in range(NT):
                cs = slice(t * D, (t + 1) * D)
                nc.gpsimd.indirect_dma_start(
                    out=ct[:, cs],
                    out_offset=None,
                    in_=centers[:],
                    in_offset=bass.IndirectOffsetOnAxis(ap=it[:, t : t + 1], axis=0),
                )
                nc.vector.tensor_tensor(
                    out=diff[:, cs], in0=xt[:, cs], in1=ct[:, cs],
                    op=mybir.AluOpType.subtract,
                )
                # Fused square + row-sum on the Scalar engine (accum_out);
                # frees the DVE for the next block's subtract.
                nc.scalar.activation(
                    out=sq[:, cs],
                    in_=diff[:, cs],
                    func=mybir.ActivationFunctionType.Square,
                    accum_out=dist[:, t : t + 1],
                )
                nc.vector.tensor_scalar(
                    out=dist[:, t : t + 1],
                    in0=dist[:, t : t + 1],
                    scalar1=CLAMP_MIN,
                    scalar2=CLAMP_MAX,
                    op0=mybir.AluOpType.max,
                    op1=mybir.AluOpType.min,
                )
            acc = psum.tile([1, NT], mybir.dt.float32)
            nc.tensor.matmul(
                out=acc[:], lhsT=ones[:], rhs=dist[:], start=True, stop=True
            )
            res = work.tile([1, NT], mybir.dt.float32)
            nc.vector.tensor_copy(out=res[:], in_=acc[:])
            nc.sync.dma_start(out=out[:], in_=res[:])
    nc.compile()
    return nc


def _marshal(x, centers, labels):
    x = np.asarray(x, dtype=np.float32)
    centers = np.ascontiguousarray(np.asarray(centers, dtype=np.float32))
    lab = np.asarray(labels).astype(np.int32).reshape(B)
    in_maps = []
    for k in range(N_CORES):
        xs = x[k * BP : (k + 1) * BP]  # [256, 128]
        ls = lab[k * BP : (k + 1) * BP]  # [256]
        # row b = n*128 + p -> partition p, column block n
        x_core = np.ascontiguousarray(
            xs.reshape(NT, P, D).transpose(1, 0, 2).reshape(P, NT * D)
        )
        idx_core = np.ascontiguousarray(ls.reshape(NT, P).T)  # [128, NT]
        in_maps.append({"x": x_core, "idx": idx_core, "centers": centers})
    return in_maps


def _run(x, centers, labels, **spmd_kwargs):
    global _NC
    if _NC is None:
        _NC = _build_nc()
    in_maps = _marshal(x, centers, labels)
    return run_bass_kernel_spmd(_NC, in_maps, list(range(N_CORES)), **spmd_kwargs)


def kernel(x, centers, labels):
    res = _run(x, centers, labels)
    total = sum(np.float64(r["out"]).sum() for r in res.results)
    return np.array(total, dtype=np.float32)


# revision 15
# speedup vs baseline: 1.0345x; 1.0345x over previous
"""CenterLoss kernel for Trainium2 (Bass/Tile), 8-core data-parallel.

loss = mean_b( clip(||x_b - centers[labels_b]||^2, 1e-12, 1e12) )

Sharding: batch (2048) split across 8 cores (256 rows each); centers
replicated (each core only *reads* its 256 gathered rows via indirect
DMA, so the 51MB table costs ~nothing in kernel time). Each core emits
[1,2] partial sums of its clipped per-row distances; the host adds the
16 partials and divides by B (the cross-core all-reduce + mean).

Layout: the host marshals each core's 256 batch rows into a single
[128, 2*128] partition-major tile (row b = n*128+p lands at partition p,
columns n*128..n*128+127) and labels into [128, 2] int32. Per column
block n: an indirect DMA gathers the 128 true-class center rows (the
HW SWDGE offset table holds one offset per partition, so 128 rows is
the per-gather max), the DVE subtracts, and the Scalar engine squares
with a fused row-sum (accum_out). Per-row clip on the DVE, then a
GPSIMD partition_all_reduce collapses the 128 partitions and one
8-byte store writes the result.

The kernel is latency-bound, not bandwidth-bound: the critical path is
preamble -> idx DMA (~2.3us fixed issue+queue+semaphore latency) ->
2x gather descriptor-gen on the Pool engine (~1us each, SWDGE fixed
overhead) -> gather tail -> short compute chain -> output store
(~2.3us fixed). Everything is merged/ordered to pay each fixed DMA
cost exactly once, idx is issued first (it gates the gathers), and
block 0's compute overlaps block 1's gather.
"""

import numpy as np

import concourse.bacc as bacc
import concourse.bass as bass
import concourse.bass_isa as bass_isa
import concourse.tile as tile
from concourse import mybir
from concourse.bass_utils import run_bass_kernel_spmd

B, C, D = 2048, 100000, 128
N_CORES = 8
BP = B // N_CORES  # 256 rows per core
P = 128  # SBUF partitions
NT = BP // P  # column blocks per core (2)
CLAMP_MIN, CLAMP_MAX = 1e-12, 1e12

_NC = None


def _build_nc():
    nc = bacc.Bacc()
    x = nc.declare_dram_parameter("x", [P, NT * D], mybir.dt.float32, isOutput=False)
    idx = nc.declare_dram_parameter("idx", [P, NT], mybir.dt.int32, isOutput=False)
    centers = nc.declare_dram_parameter(
        "centers", [C, D], mybir.dt.float32, isOutput=False
    )
    out = nc.declare_dram_parameter("out", [1, NT], mybir.dt.float32, isOutput=True)

    with tile.TileContext(nc) as tc:
        with tc.tile_pool(name="work", bufs=1) as work:
            it = work.tile([P, NT], mybir.dt.int32)
            nc.sync.dma_start(out=it[:], in_=idx[:, :])
            xt = work.tile([P, NT * D], mybir.dt.float32)
            nc.sync.dma_start(out=xt[:], in_=x[:, :])

            ct = work.tile([P, NT * D], mybir.dt.float32)
            diff = work.tile([P, NT * D], mybir.dt.float32)
            sq = work.tile([P, NT * D], mybir.dt.float32)
            dist = work.tile([P, NT], mybir.dt.float32)
            for t in range(NT):
                cs = slice(t * D, (t + 1) * D)
                nc.gpsimd.indirect_dma_start(
                    out=ct[:, cs],
                    out_offset=None,
                    in_=centers[:],
                    in_offset=bass.IndirectOffsetOnAxis(ap=it[:, t : t + 1], axis=0),
                )
                nc.vector.tensor_tensor(
                    out=diff[:, cs], in0=xt[:, cs], in1=ct[:, cs],
                    op=mybir.AluOpType.subtract,
                )
                # Fused square + row-sum on the Scalar engine (accum_out);
                # frees the DVE for the next block's subtract.
                nc.scalar.activation(
                    out=sq[:, cs],
                    in_=diff[:, cs],
                    func=mybir.ActivationFunctionType.Square,
                    accum_out=dist[:, t : t + 1],
                )
                nc.vector.tensor_scalar(
                    out=dist[:, t : t + 1],
                    in0=dist[:, t : t + 1],
                    scalar1=CLAMP_MIN,
                    scalar2=CLAMP_MAX,
                    op0=mybir.AluOpType.max,
                    op1=mybir.AluOpType.min,
                )
            red = work.tile([P, NT], mybir.dt.float32)
            nc.gpsimd.partition_all_reduce(
                red[:], dist[:], channels=P, reduce_op=bass_isa.ReduceOp.add
            )
            nc.sync.dma_start(out=out[:], in_=red[:1, :])
    nc.compile()
    return nc


def _marshal(x, centers, labels):
    x = np.asarray(x, dtype=np.float32)
    centers = np.ascontiguousarray(np.asarray(centers, dtype=np.float32))
    lab = np.asarray(labels).astype(np.int32).reshape(B)
    in_maps = []
    for k in range(N_CORES):
        xs = x[k * BP : (k + 1) * BP]  # [256, 128]
        ls = lab[k * BP : (k + 1) * BP]  # [256]
        # row b = n*128 + p -> partition p, column block n
        x_core = np.ascontiguousarray(
            xs.reshape(NT, P, D).transpose(1, 0, 2).reshape(P, NT * D)
        )
        idx_core = np.ascontiguousarray(ls.reshape(NT, P).T)  # [128, NT]
        in_maps.append({"x": x_core, "idx": idx_core, "centers": centers})
    return in_maps


def _run(x, centers, labels, **spmd_kwargs):
    global _NC
    if _NC is None:
        _NC = _build_nc()
    in_maps = _marshal(x, centers, labels)
    return run_bass_kernel_spmd(_NC, in_maps, list(range(N_CORES)), **spmd_kwargs)


def kernel(x, centers, labels):
    res = _run(x, centers, labels)
    total = sum(np.float64(r["out"]).sum() for r in res.results)
    return np.array(total / B, dtype=np.float32)


# revision 18
# speedup vs baseline: 1.0723x; 1.0365x over previous
"""CenterLoss kernel for Trainium2 (Bass/Tile), 8-core data-parallel.

loss = mean_b( clip(||x_b - centers[labels_b]||^2, 1e-12, 1e12) )

Sharding: batch (2048) split across 8 cores (256 rows each); centers
replicated (each core only *reads* its 256 gathered rows via indirect
DMA, so the 51MB table costs ~nothing in kernel time). Each core emits
[1,2] partial sums of its clipped per-row distances; the host adds the
16 partials and divides by B (the cross-core all-reduce + mean).

Layout: the host marshals each core's 256 batch rows into a single
[128, 2*128] partition-major tile (row b = n*128+p lands at partition p,
columns n*128..n*128+127) and labels into [128, 2] int32. Per column
block n: an indirect DMA gathers the 128 true-class center rows (the
HW SWDGE offset table holds one offset per partition, so 128 rows is
the per-gather max), the DVE subtracts, and the Scalar engine squares
with a fused row-sum (accum_out). Per-row clip on the DVE, then a
GPSIMD partition_all_reduce collapses the 128 partitions and one
8-byte store writes the result.

The kernel is latency-bound, not bandwidth-bound: the critical path is
preamble -> idx DMA (~2.3us fixed issue+queue+semaphore latency) ->
2x gather descriptor-gen on the Pool engine (~1us each, SWDGE fixed
overhead) -> gather tail -> short compute chain -> output store
(~2.3us fixed). Everything is merged/ordered to pay each fixed DMA
cost exactly once, idx is issued first (it gates the gathers), and
block 0's compute overlaps block 1's gather.
"""

import numpy as np

import concourse.bacc as bacc
import concourse.bass as bass
import concourse.bass_isa as bass_isa
import concourse.tile as tile
from concourse import mybir
from concourse.bass_utils import run_bass_kernel_spmd

B, C, D = 2048, 100000, 128
N_CORES = 8
BP = B // N_CORES  # 256 rows per core
P = 128  # SBUF partitions
NT = BP // P  # column blocks per core (2)
CLAMP_MIN, CLAMP_MAX = 1e-12, 1e12

_NC = None


def _build_nc():
    nc = bacc.Bacc()
    x = nc.declare_dram_parameter("x", [P, NT * D], mybir.dt.float32, isOutput=False)
    idx = nc.declare_dram_parameter("idx", [P, NT], mybir.dt.int32, isOutput=False)
    centers = nc.declare_dram_parameter(
        "centers", [C, D], mybir.dt.float32, isOutput=False
    )
    out = nc.declare_dram_parameter("out", [1, NT], mybir.dt.float32, isOutput=True)

    with tile.TileContext(nc) as tc:
        with tc.tile_pool(name="work", bufs=1) as work:
            it = work.tile([P, NT], mybir.dt.int32)
            nc.sync.dma_start(out=it[:], in_=idx[:, :])
            xt = work.tile([P, NT * D], mybir.dt.float32)
            nc.sync.dma_start(out=xt[:], in_=x[:, :])
            # Explicit zero bias for the Square activation so nothing reads
            # the Bass preamble const tensors (stripped below).
            zbias = work.tile([P, 1], mybir.dt.float32)
            nc.vector.memset(zbias[:], 0.0)

            ct = work.tile([P, NT * D], mybir.dt.float32)
            diff = work.tile([P, NT * D], mybir.dt.float32)
            sq = work.tile([P, NT * D], mybir.dt.float32)
            dist = work.tile([P, NT], mybir.dt.float32)
            for t in range(NT):
                cs = slice(t * D, (t + 1) * D)
                nc.gpsimd.indirect_dma_start(
                    out=ct[:, cs],
                    out_offset=None,
                    in_=centers[:],
                    in_offset=bass.IndirectOffsetOnAxis(ap=it[:, t : t + 1], axis=0),
                )
                nc.vector.tensor_tensor(
                    out=diff[:, cs], in0=xt[:, cs], in1=ct[:, cs],
                    op=mybir.AluOpType.subtract,
                )
                # Fused square + row-sum on the Scalar engine (accum_out);
                # frees the DVE for the next block's subtract.
                nc.scalar.activation(
                    out=sq[:, cs],
                    in_=diff[:, cs],
                    func=mybir.ActivationFunctionType.Square,
                    bias=zbias[:, :1],
                    accum_out=dist[:, t : t + 1],
                )
                nc.vector.tensor_scalar(
                    out=dist[:, t : t + 1],
                    in0=dist[:, t : t + 1],
                    scalar1=CLAMP_MIN,
                    scalar2=CLAMP_MAX,
                    op0=mybir.AluOpType.max,
                    op1=mybir.AluOpType.min,
                )
            red = work.tile([P, NT], mybir.dt.float32)
            nc.gpsimd.partition_all_reduce(
                red[:], dist[:], channels=P, reduce_op=bass_isa.ReduceOp.add
            )
            nc.sync.dma_start(out=out[:], in_=red[:1, :])
    # The Bass preamble unconditionally memsets four const tensors on the
    # Pool engine *before* the all-engine barrier, gating kernel start by
    # ~400ns. Nothing in this kernel reads them (the Square's bias is the
    # explicit zbias tile), so drop those memsets; bacc's
    # remove_dangling_data cleans up the now-unused allocations.
    for blk in nc.m.functions[0].blocks:
        blk.instructions = [
            inst
            for inst in blk.instructions
            if not (
                type(inst).__name__ == "InstMemset"
                and inst.outs
                and str(getattr(inst.outs[0], "memref", "")).startswith("const-")
            )
        ]
    nc.compile()
    return nc


def _marshal(x, centers, labels):
    x = np.asarray(x, dtype=np.float32)
    centers = np.ascontiguousarray(np.asarray(centers, dtype=np.float32))
    lab = np.asarray(labels).astype(np.int32).reshape(B)
    in_maps = []
    for k in range(N_CORES):
        xs = x[k * BP : (k + 1) * BP]  # [256, 128]
        ls = lab[k * BP : (k + 1) * BP]  # [256]
        # row b = n*128 + p -> partition p, column block n
        x_core = np.ascontiguousarray(
            xs.reshape(NT, P, D).transpose(1, 0, 2).reshape(P, NT * D)
        )
        idx_core = np.ascontiguousarray(ls.reshape(NT, P).T)  # [128, NT]
        in_maps.append({"x": x_core, "idx": idx_core, "centers": centers})
    return in_maps


def _run(x, centers, labels, **spmd_kwargs):
    global _NC
    if _NC is None:
        _NC = _build_nc()
    in_maps = _marshal(x, centers, labels)
    return run_bass_kernel_spmd(_NC, in_maps, list(range(N_CORES)), **spmd_kwargs)


def kernel(x, centers, labels):
    res = _run(x, centers, labels)
    total = sum(np.float64(r["out"]).sum() for r in res.results)
    return np.array(total / B, dtype=np.float32)


# revision 19
# speedup vs baseline: 1.0996x; 1.0254x over previous
"""CenterLoss kernel for Trainium2 (Bass/Tile), 8-core data-parallel.

loss = mean_b( clip(||x_b - centers[labels_b]||^2, 1e-12, 1e12) )

Sharding: batch (2048) split across 8 cores (256 rows each); centers
replicated (each core only *reads* its 256 gathered rows via indirect
DMA, so the 51MB table costs ~nothing in kernel time). Each core emits
[1,2] partial sums of its clipped per-row distances; the host adds the
16 partials and divides by B (the cross-core all-reduce + mean).

Layout: the host marshals each core's 256 batch rows into a single
[128, 2*128] partition-major tile (row b = n*128+p lands at partition p,
columns n*128..n*128+127) and labels into [128, 2] int32. Per column
block n: an indirect DMA gathers the 128 true-class center rows (the
HW SWDGE offset table holds one offset per partition, so 128 rows is
the per-gather max), the DVE subtracts, and the Scalar engine squares
with a fused row-sum (accum_out). Per-row clip on the DVE, then a
GPSIMD partition_all_reduce collapses the 128 partitions and one
8-byte store writes the result.

The kernel is latency-bound, not bandwidth-bound: the critical path is
preamble -> idx DMA (~2.3us fixed issue+queue+semaphore latency) ->
2x gather descriptor-gen on the Pool engine (~1us each, SWDGE fixed
overhead) -> gather tail -> short compute chain -> output store
(~2.3us fixed). Everything is merged/ordered to pay each fixed DMA
cost exactly once, idx is issued first (it gates the gathers), and
block 0's compute overlaps block 1's gather.
"""

import numpy as np

import concourse.bacc as bacc
import concourse.bass as bass
import concourse.bass_isa as bass_isa
import concourse.tile as tile
from concourse import mybir
from concourse.bass_utils import run_bass_kernel_spmd

B, C, D = 2048, 100000, 128
N_CORES = 8
BP = B // N_CORES  # 256 rows per core
P = 128  # SBUF partitions
NT = BP // P  # column blocks per core (2)
CLAMP_MIN, CLAMP_MAX = 1e-12, 1e12

_NC = None


def _build_nc():
    nc = bacc.Bacc()
    x = nc.declare_dram_parameter("x", [P, NT * D], mybir.dt.float32, isOutput=False)
    idx = nc.declare_dram_parameter("idx", [P, NT], mybir.dt.int32, isOutput=False)
    centers = nc.declare_dram_parameter(
        "centers", [C, D], mybir.dt.float32, isOutput=False
    )
    out = nc.declare_dram_parameter("out", [1, NT], mybir.dt.float32, isOutput=True)

    with tile.TileContext(nc) as tc:
        with tc.tile_pool(name="work", bufs=1) as work:
            it = work.tile([P, NT], mybir.dt.int32)
            nc.sync.dma_start(out=it[:], in_=idx[:, :])
            xt = work.tile([P, NT * D], mybir.dt.float32)
            nc.sync.dma_start(out=xt[:], in_=x[:, :])
            # Explicit zero bias for the Square activation so nothing reads
            # the Bass preamble const tensors (stripped below).
            zbias = work.tile([P, 1], mybir.dt.float32)
            nc.vector.memset(zbias[:], 0.0)

            ct = work.tile([P, NT * D], mybir.dt.float32)
            diff = work.tile([P, NT * D], mybir.dt.float32)
            sq = work.tile([P, NT * D], mybir.dt.float32)
            dist = work.tile([P, NT], mybir.dt.float32)
            for t in range(NT):
                cs = slice(t * D, (t + 1) * D)
                nc.gpsimd.indirect_dma_start(
                    out=ct[:, cs],
                    out_offset=None,
                    in_=centers[:],
                    in_offset=bass.IndirectOffsetOnAxis(ap=it[:, t : t + 1], axis=0),
                )
                nc.vector.tensor_tensor(
                    out=diff[:, cs], in0=xt[:, cs], in1=ct[:, cs],
                    op=mybir.AluOpType.subtract,
                )
                # Fused square + row-sum on the Scalar engine (accum_out);
                # frees the DVE for the next block's subtract.
                nc.scalar.activation(
                    out=sq[:, cs],
                    in_=diff[:, cs],
                    func=mybir.ActivationFunctionType.Square,
                    bias=zbias[:, :1],
                    accum_out=dist[:, t : t + 1],
                )
                nc.vector.tensor_scalar(
                    out=dist[:, t : t + 1],
                    in0=dist[:, t : t + 1],
                    scalar1=CLAMP_MIN,
                    scalar2=CLAMP_MAX,
                    op0=mybir.AluOpType.max,
                    op1=mybir.AluOpType.min,
                )
            red = work.tile([P, NT], mybir.dt.float32)
            nc.gpsimd.partition_all_reduce(
                red[:], dist[:], channels=P, reduce_op=bass_isa.ReduceOp.add
            )
            nc.sync.dma_start(out=out[:], in_=red[:1, :])
    # The Bass preamble unconditionally memsets four const tensors on the
    # Pool engine and then runs an all-engine barrier, gating kernel start
    # by ~650ns. The barrier exists only to order those memsets before any
    # const reader; nothing in this kernel reads them (the Square's bias is
    # the explicit zbias tile), so drop both the memsets and the preamble
    # barrier. Every remaining cross-engine dependency is an explicit
    # Tile-emitted semaphore counted from 0, and the BSP exit sequence
    # resets all semaphores, so repeat executions stay correct (verified by
    # back-to-back runs). bacc's remove_dangling_data cleans up the unused
    # const allocations.
    for blk in nc.m.functions[0].blocks:
        keep = []
        for inst in blk.instructions:
            tn = type(inst).__name__
            if (
                tn == "InstMemset"
                and inst.outs
                and str(getattr(inst.outs[0], "memref", "")).startswith("const-")
            ):
                continue
            if blk.name == "main" and tn in ("InstDrain", "InstEventSemaphore"):
                continue
            keep.append(inst)
        blk.instructions = keep
    nc.compile()
    return nc


def _marshal(x, centers, labels):
    x = np.asarray(x, dtype=np.float32)
    centers = np.ascontiguousarray(np.asarray(centers, dtype=np.float32))
    lab = np.asarray(labels).astype(np.int32).reshape(B)
    in_maps = []
    for k in range(N_CORES):
        xs = x[k * BP : (k + 1) * BP]  # [256, 128]
        ls = lab[k * BP : (k + 1) * BP]  # [256]
        # row b = n*128 + p -> partition p, column block n
        x_core = np.ascontiguousarray(
            xs.reshape(NT, P, D).transpose(1, 0, 2).reshape(P, NT * D)
        )
        idx_core = np.ascontiguousarray(ls.reshape(NT, P).T)  # [128, NT]
        in_maps.append({"x": x_core, "idx": idx_core, "centers": centers})
    return in_maps


def _run(x, centers, labels, **spmd_kwargs):
    global _NC
    if _NC is None:
        _NC = _build_nc()
    in_maps = _marshal(x, centers, labels)
    return run_bass_kernel_spmd(_NC, in_maps, list(range(N_CORES)), **spmd_kwargs)


def kernel(x, centers, labels):
    res = _run(x, centers, labels)
    total = sum(np.float64(r["out"]).sum() for r in res.results)
    return np.array(total / B, dtype=np.float32)


# revision 20
# speedup vs baseline: 1.1296x; 1.0273x over previous
"""CenterLoss kernel for Trainium2 (Bass/Tile), 8-core data-parallel.

loss = mean_b( clip(||x_b - centers[labels_b]||^2, 1e-12, 1e12) )

Sharding: batch (2048) split across 8 cores (256 rows each); centers
replicated (each core only *reads* its 256 gathered rows via indirect
DMA, so the 51MB table costs ~nothing in kernel time). Each core emits
[1,2] partial sums of its clipped per-row distances; the host adds the
16 partials and divides by B (the cross-core all-reduce + mean).

Layout: the host marshals each core's 256 batch rows into a single
[128, 2*128] partition-major tile (row b = n*128+p lands at partition p,
columns n*128..n*128+127) and labels into [128, 2] int32. Per column
block n: an indirect DMA gathers the 128 true-class center rows (the
HW SWDGE offset table holds one offset per partition, so 128 rows is
the per-gather max), the DVE subtracts, and the Scalar engine squares
with a fused row-sum (accum_out). Per-row clip on the DVE, then a
GPSIMD partition_all_reduce collapses the 128 partitions and one
8-byte store writes the result.

The kernel is latency-bound, not bandwidth-bound: the critical path is
preamble -> idx DMA (~2.3us fixed issue+queue+semaphore latency) ->
2x gather descriptor-gen on the Pool engine (~1us each, SWDGE fixed
overhead) -> gather tail -> short compute chain -> output store
(~2.3us fixed). Everything is merged/ordered to pay each fixed DMA
cost exactly once, idx is issued first (it gates the gathers), and
block 0's compute overlaps block 1's gather.
"""

import numpy as np

import concourse.bacc as bacc
import concourse.bass as bass
import concourse.bass_isa as bass_isa
import concourse.tile as tile
from concourse import mybir
from concourse.bass_utils import run_bass_kernel_spmd

B, C, D = 2048, 100000, 128
N_CORES = 8
BP = B // N_CORES  # 256 rows per core
P = 128  # SBUF partitions
NT = BP // P  # column blocks per core (2)
CLAMP_MIN, CLAMP_MAX = 1e-12, 1e12

_NC = None


def _build_nc():
    nc = bacc.Bacc()
    x = nc.declare_dram_parameter("x", [P, NT * D], mybir.dt.float32, isOutput=False)
    idx = nc.declare_dram_parameter("idx", [P, NT], mybir.dt.int32, isOutput=False)
    centers = nc.declare_dram_parameter(
        "centers", [C, D], mybir.dt.float32, isOutput=False
    )
    out = nc.declare_dram_parameter("out", [1, NT], mybir.dt.float32, isOutput=True)

    with tile.TileContext(nc) as tc:
        with tc.tile_pool(name="work", bufs=1) as work:
            it = work.tile([P, NT], mybir.dt.int32)
            nc.sync.dma_start(out=it[:], in_=idx[:, :])
            xt = work.tile([P, NT * D], mybir.dt.float32)
            nc.sync.dma_start(out=xt[:], in_=x[:, :])
            # Explicit zero bias for the Square activation so nothing reads
            # the Bass preamble const tensors (stripped below).
            zbias = work.tile([P, 1], mybir.dt.float32)
            nc.vector.memset(zbias[:], 0.0)

            ct = work.tile([P, NT * D], mybir.dt.float32)
            diff = work.tile([P, NT * D], mybir.dt.float32)
            sq = work.tile([P, NT * D], mybir.dt.float32)
            dist = work.tile([P, NT], mybir.dt.float32)
            for t in range(NT):
                cs = slice(t * D, (t + 1) * D)
                nc.gpsimd.indirect_dma_start(
                    out=ct[:, cs],
                    out_offset=None,
                    in_=centers[:],
                    in_offset=bass.IndirectOffsetOnAxis(ap=it[:, t : t + 1], axis=0),
                )
                nc.vector.tensor_tensor(
                    out=diff[:, cs], in0=xt[:, cs], in1=ct[:, cs],
                    op=mybir.AluOpType.subtract,
                )
                # Fused square + row-sum on the Scalar engine (accum_out);
                # frees the DVE for the next block's subtract.
                nc.scalar.activation(
                    out=sq[:, cs],
                    in_=diff[:, cs],
                    func=mybir.ActivationFunctionType.Square,
                    bias=zbias[:, :1],
                    accum_out=dist[:, t : t + 1],
                )
                nc.vector.tensor_scalar(
                    out=dist[:, t : t + 1],
                    in0=dist[:, t : t + 1],
                    scalar1=CLAMP_MIN,
                    scalar2=CLAMP_MAX,
                    op0=mybir.AluOpType.max,
                    op1=mybir.AluOpType.min,
                )
            red = work.tile([P, NT], mybir.dt.float32)
            nc.gpsimd.partition_all_reduce(
                red[:], dist[:], channels=P, reduce_op=bass_isa.ReduceOp.add
            )
            nc.sync.dma_start(out=out[:], in_=red[:1, :])
    # The Bass preamble unconditionally memsets four const tensors on the
    # Pool engine and then runs an all-engine barrier, gating kernel start
    # by ~650ns. The barrier exists only to order those memsets before any
    # const reader; nothing in this kernel reads them (the Square's bias is
    # the explicit zbias tile), so drop both the memsets and the preamble
    # barrier. Every remaining cross-engine dependency is an explicit
    # Tile-emitted semaphore counted from 0, and the BSP exit sequence
    # resets all semaphores, so repeat executions stay correct (verified by
    # back-to-back runs). bacc's remove_dangling_data cleans up the unused
    # const allocations.
    # Also strip the exit sequence's second all-engine butterfly (the one
    # AFTER the semaphore-reset ISA op): engines have no semaphore uses
    # after the first butterfly, and Pool halts only after completing the
    # reset, so execution-complete still implies sems are reset.
    for blk in nc.m.functions[0].blocks:
        keep = []
        seen_isa = False
        for inst in blk.instructions:
            tn = type(inst).__name__
            if (
                tn == "InstMemset"
                and inst.outs
                and str(getattr(inst.outs[0], "memref", "")).startswith("const-")
            ):
                continue
            if blk.name == "main" and tn in ("InstDrain", "InstEventSemaphore"):
                continue
            if tn == "InstISA":
                seen_isa = True
            if (
                blk.name.endswith("_end")
                and seen_isa
                and tn in ("InstDrain", "InstEventSemaphore")
            ):
                continue
            keep.append(inst)
        blk.instructions = keep
    nc.compile()
    return nc


def _marshal(x, centers, labels):
    x = np.asarray(x, dtype=np.float32)
    centers = np.ascontiguousarray(np.asarray(centers, dtype=np.float32))
    lab = np.asarray(labels).astype(np.int32).reshape(B)
    in_maps = []
    for k in range(N_CORES):
        xs = x[k * BP : (k + 1) * BP]  # [256, 128]
        ls = lab[k * BP : (k + 1) * BP]  # [256]
        # row b = n*128 + p -> partition p, column block n
        x_core = np.ascontiguousarray(
            xs.reshape(NT, P, D).transpose(1, 0, 2).reshape(P, NT * D)
        )
        idx_core = np.ascontiguousarray(ls.reshape(NT, P).T)  # [128, NT]
        in_maps.append({"x": x_core, "idx": idx_core, "centers": centers})
    return in_maps


def _run(x, centers, labels, **spmd_kwargs):
    global _NC
    if _NC is None:
        _NC = _build_nc()
    in_maps = _marshal(x, centers, labels)
    return run_bass_kernel_spmd(_NC, in_maps, list(range(N_CORES)), **spmd_kwargs)


def kernel(x, centers, labels):
    res = _run(x, centers, labels)
    total = sum(np.float64(r["out"]).sum() for r in res.results)
    return np.array(total / B, dtype=np.float32)


# revision 21
# speedup vs baseline: 1.1517x; 1.0196x over previous
"""CenterLoss kernel for Trainium2 (Bass/Tile), 8-core data-parallel.

loss = mean_b( clip(||x_b - centers[labels_b]||^2, 1e-12, 1e12) )

Sharding: batch (2048) split across 8 cores (256 rows each); centers
replicated (each core only *reads* its 256 gathered rows via indirect
DMA, so the 51MB table costs ~nothing in kernel time). Each core emits
[1,2] partial sums of its clipped per-row distances; the host adds the
16 partials and divides by B (the cross-core all-reduce + mean).

Layout: the host marshals each core's 256 batch rows into a single
[128, 2*128] partition-major tile (row b = n*128+p lands at partition p,
columns n*128..n*128+127) and labels into [128, 2] int32. Per column
block n: an indirect DMA gathers the 128 true-class center rows (the
HW SWDGE offset table holds one offset per partition, so 128 rows is
the per-gather max), the DVE subtracts, and the Scalar engine squares
with a fused row-sum (accum_out). Per-row clip on the DVE, then a
GPSIMD partition_all_reduce collapses the 128 partitions and one
8-byte store writes the result.

The kernel is latency-bound, not bandwidth-bound: the critical path is
preamble -> idx DMA (~2.3us fixed issue+queue+semaphore latency) ->
2x gather descriptor-gen on the Pool engine (~1us each, SWDGE fixed
overhead) -> gather tail -> short compute chain -> output store
(~2.3us fixed). Everything is merged/ordered to pay each fixed DMA
cost exactly once, idx is issued first (it gates the gathers), and
block 0's compute overlaps block 1's gather.
"""

import numpy as np

import concourse.bacc as bacc
import concourse.bass as bass
import concourse.bass_isa as bass_isa
import concourse.tile as tile
from concourse import mybir
from concourse.bass_utils import run_bass_kernel_spmd

B, C, D = 2048, 100000, 128
N_CORES = 8
BP = B // N_CORES  # 256 rows per core
P = 128  # SBUF partitions
NT = BP // P  # column blocks per core (2)
CLAMP_MIN, CLAMP_MAX = 1e-12, 1e12

_NC = None


def _build_nc():
    nc = bacc.Bacc()
    x = nc.declare_dram_parameter("x", [P, NT * D], mybir.dt.float32, isOutput=False)
    idx = nc.declare_dram_parameter("idx", [P, NT], mybir.dt.int32, isOutput=False)
    centers = nc.declare_dram_parameter(
        "centers", [C, D], mybir.dt.float32, isOutput=False
    )
    out = nc.declare_dram_parameter("out", [1, NT], mybir.dt.float32, isOutput=True)

    with tile.TileContext(nc) as tc:
        with tc.tile_pool(name="work", bufs=1) as work:
            it = work.tile([P, NT], mybir.dt.int32)
            nc.sync.dma_start(out=it[:], in_=idx[:, :])
            xt = work.tile([P, NT * D], mybir.dt.float32)
            nc.sync.dma_start(out=xt[:], in_=x[:, :])
            # Explicit zero bias for the Square activation so nothing reads
            # the Bass preamble const tensors (stripped below).
            zbias = work.tile([P, 1], mybir.dt.float32)
            nc.vector.memset(zbias[:], 0.0)

            ct = work.tile([P, NT * D], mybir.dt.float32)
            diff = work.tile([P, NT * D], mybir.dt.float32)
            sq = work.tile([P, NT * D], mybir.dt.float32)
            dist = work.tile([P, NT], mybir.dt.float32)
            for t in range(NT):
                cs = slice(t * D, (t + 1) * D)
                nc.gpsimd.indirect_dma_start(
                    out=ct[:, cs],
                    out_offset=None,
                    in_=centers[:],
                    in_offset=bass.IndirectOffsetOnAxis(ap=it[:, t : t + 1], axis=0),
                )
                nc.vector.tensor_tensor(
                    out=diff[:, cs], in0=xt[:, cs], in1=ct[:, cs],
                    op=mybir.AluOpType.subtract,
                )
                # Fused square + row-sum on the Scalar engine (accum_out);
                # frees the DVE for the next block's subtract.
                nc.scalar.activation(
                    out=sq[:, cs],
                    in_=diff[:, cs],
                    func=mybir.ActivationFunctionType.Square,
                    bias=zbias[:, :1],
                    accum_out=dist[:, t : t + 1],
                )
                nc.vector.tensor_scalar(
                    out=dist[:, t : t + 1],
                    in0=dist[:, t : t + 1],
                    scalar1=CLAMP_MIN,
                    scalar2=CLAMP_MAX,
                    op0=mybir.AluOpType.max,
                    op1=mybir.AluOpType.min,
                )
            red = work.tile([P, NT], mybir.dt.float32)
            nc.gpsimd.partition_all_reduce(
                red[:], dist[:], channels=P, reduce_op=bass_isa.ReduceOp.add
            )
            nc.sync.dma_start(out=out[:], in_=red[:1, :])
    # The Bass preamble unconditionally memsets four const tensors on the
    # Pool engine and then runs an all-engine barrier, gating kernel start
    # by ~650ns. The barrier exists only to order those memsets before any
    # const reader; nothing in this kernel reads them (the Square's bias is
    # the explicit zbias tile), so drop both the memsets and the preamble
    # barrier. Every remaining cross-engine dependency is an explicit
    # Tile-emitted semaphore counted from 0, and the BSP exit sequence
    # resets all semaphores, so repeat executions stay correct (verified by
    # back-to-back runs). bacc's remove_dangling_data cleans up the unused
    # const allocations.
    # Exit-sequence surgery (each step HW-verified over repeated runs):
    # stock BSP exit is [all-sem Drain on SP] -> [EVSEM butterfly] ->
    # [sem-range-clear ISA on Pool] -> [second butterfly] -> halt. The
    # butterflies only exist to order the Pool-issued reset against other
    # engines' in-flight semaphore updates. Moving the reset ISA to SP --
    # whose all-sem Drain has already observed every final semaphore value,
    # proving all updates landed -- makes both butterflies redundant: drop
    # every exit EventSemaphore, the post-reset tail, and the duplicate SP
    # drain. Per-engine pipeline Drains are kept.
    for blk in nc.m.functions[0].blocks:
        keep = []
        seen_isa = False
        kept_sp_drain = False
        for inst in blk.instructions:
            tn = type(inst).__name__
            if (
                tn == "InstMemset"
                and inst.outs
                and str(getattr(inst.outs[0], "memref", "")).startswith("const-")
            ):
                continue
            if blk.name == "main" and tn in ("InstDrain", "InstEventSemaphore"):
                continue
            if tn == "InstISA":
                seen_isa = True
                inst.engine = mybir.EngineType.SP
            if blk.name.endswith("_end"):
                if seen_isa and tn in ("InstDrain", "InstEventSemaphore"):
                    continue
                if tn == "InstEventSemaphore":
                    continue
                if tn == "InstDrain" and inst.engine == mybir.EngineType.SP:
                    if kept_sp_drain:
                        continue
                    kept_sp_drain = True
            keep.append(inst)
        blk.instructions = keep
    nc.compile()
    return nc


def _marshal(x, centers, labels):
    x = np.asarray(x, dtype=np.float32)
    centers = np.ascontiguousarray(np.asarray(centers, dtype=np.float32))
    lab = np.asarray(labels).astype(np.int32).reshape(B)
    in_maps = []
    for k in range(N_CORES):
        xs = x[k * BP : (k + 1) * BP]  # [256, 128]
        ls = lab[k * BP : (k + 1) * BP]  # [256]
        # row b = n*128 + p -> partition p, column block n
        x_core = np.ascontiguousarray(
            xs.reshape(NT, P, D).transpose(1, 0, 2).reshape(P, NT * D)
        )
        idx_core = np.ascontiguousarray(ls.reshape(NT, P).T)  # [128, NT]
        in_maps.append({"x": x_core, "idx": idx_core, "centers": centers})
    return in_maps


def _run(x, centers, labels, **spmd_kwargs):
    global _NC
    if _NC is None:
        _NC = _build_nc()
    in_maps = _marshal(x, centers, labels)
    return run_bass_kernel_spmd(_NC, in_maps, list(range(N_CORES)), **spmd_kwargs)


def kernel(x, centers, labels):
    res = _run(x, centers, labels)
    total = sum(np.float64(r["out"]).sum() for r in res.results)
    return np.array(total / B, dtype=np.float32)
